# revision 12
# baseline (speedup 1.0000x reference)
"""Trainium2 Bass kernel for nn_MemoryAugmented (scatter_memory).

Computes, for full inputs x:[64,12,883,64], M:[12,64,64]:
    score = softmax(einsum('blnd,tmd->btnm', x, M), axis=-1)
    out   = einsum('btnm,tmd->btnd', score, M)

Distribution: data-parallel over batch across 8 NeuronCores (8 batches
per core); the memory bank M is replicated, shipped pre-transformed into
two constant matrices (paired-t M^T for mm1, block-diagonal M + ones
columns for mm2's fused row sums).

Precision: x and M travel as fp16 (matmuls run at 1 cycle/row vs 4 for
fp32, HBM traffic halves); exp values are bf16 (need fp32-like range);
PSUM accumulation is always fp32; output is stored fp16 and upcast on
the host. Measured end-to-end max rel err ~4e-3 vs the 2e-2 gate.

Per-core dataflow, 7 iterations of 1024 rows r = (b, n), software-
pipelined one deep so no engine waits on another's latest result:
  body(it): load(it+1); l-sum tree(it+1) [L1 on gpsimd, L2-4 on DVE];
  for each t-pair: mm1(it) x2 + exp(it) x2 interleaved with one value
  chunk of mm2(it-1) + reciprocal + normalize (DVE); remaining chunks;
  store(it-1); PE transposes + ACT copies build xsT(it+1) last.
"""
import sys

for _p in ("/opt/trn_rl_repo",):
    if _p not in sys.path:
        sys.path.insert(0, _p)

from contextlib import ExitStack

import numpy as np

import concourse.bass as bass
import concourse.bacc as bacc
import concourse.tile as tile
from concourse import mybir
from concourse._compat import with_exitstack
from concourse.bass_utils import run_bass_kernel_spmd

B, L, N, D = 64, 12, 883, 64
T, MNUM = 12, 64
NCORES = 8
BS = B // NCORES          # 8 batches per core
NPAD = 896                # per-batch row pad (7*128)
ROWS = BS * NPAD          # 7168 rows per core
NIT = 7                   # iterations of 1024 rows
F32 = mybir.dt.float32
F16 = mybir.dt.float16
BF16 = mybir.dt.bfloat16


def build_consts(M):
    """Host-side layout prep (pure data movement) of the memory bank."""
    M = np.asarray(M, dtype=np.float32)
    mt2h = np.zeros((64, 6 * 128), np.float16)   # [d, (tp, q, m)] = M[2tp+q].T
    mbd = np.zeros((128, 6 * 130), np.float32)   # [(q, m), (tp, q, d | sums)]
    for tp in range(6):
        t0, t1 = 2 * tp, 2 * tp + 1
        mt2h[:, tp * 128 + 0:tp * 128 + 64] = M[t0].T.astype(np.float16)
        mt2h[:, tp * 128 + 64:tp * 128 + 128] = M[t1].T.astype(np.float16)
        mbd[0:64, tp * 130 + 0:tp * 130 + 64] = M[t0]
        mbd[64:128, tp * 130 + 64:tp * 130 + 128] = M[t1]
        mbd[0:64, tp * 130 + 128] = 1.0
        mbd[64:128, tp * 130 + 129] = 1.0
    # mirrored into both partition halves: parity-1 matmuls read their
    # stationary from partitions 64:128 (row group h1)
    mt2h2 = np.concatenate([mt2h, mt2h], axis=0)
    eye = np.eye(128, dtype=np.float16)
    return mt2h2, mbd, eye


@with_exitstack
def kernel_body(ctx: ExitStack, tc: "tile.TileContext", out: bass.AP,
                x: bass.AP, mt2h: bass.AP, mbd: bass.AP, eye: bass.AP):
    nc = tc.nc
    consts = ctx.enter_context(tc.tile_pool(name="consts", bufs=1))
    work = ctx.enter_context(tc.tile_pool(name="work", bufs=2))
    psum = ctx.enter_context(tc.tile_pool(name="psum", bufs=1, space="PSUM"))

    # const loads ride the scalar HWDGE ring (idle at kernel start) so the
    # first x-load isn't queued behind them on the sync ring's FIFO.
    mt2h_sb = consts.tile([128, 6 * 128], F16)
    nc.scalar.dma_start(out=mt2h_sb[:], in_=mt2h[:])
    mbd_sb = consts.tile([128, 6 * 130], BF16)
    nc.scalar.dma_start(out=mbd_sb[:], in_=mbd[:])
    eye_sb = consts.tile([128, 128], F16)
    nc.scalar.dma_start(out=eye_sb[:], in_=eye[:])
    zbias = consts.tile([128, 1], F32)
    nc.vector.memset(zbias[:], 0.0)

    def load(it):
        # one 1.5 MB load; partition p <- rows 8p..8p+7 (12 KB contiguous)
        xt = work.tile([128, 8 * L * D], F16, tag="xt", bufs=3)
        nc.sync.dma_start(
            out=xt[:].rearrange("p (c f) -> p c f", c=8),
            in_=x[1024 * it:1024 * it + 1024, :, :]
                .rearrange("(p c) l d -> p c (l d)", c=8),
        )
        return xt

    def tree_l1(xt):
        # 12 -> 6 on gpsimd (otherwise idle); the rest on DVE in fp16 2x
        t384 = work.tile([128, 8 * 384], F16, tag="t384", bufs=2)
        xtv = xt[:].rearrange("p (c h f) -> p c h f", c=8, h=2)
        nc.gpsimd.tensor_add(t384[:].rearrange("p (c f) -> p c f", c=8),
                             xtv[:, :, 0], xtv[:, :, 1])
        return t384

    def tree_rest(t384):
        t192 = work.tile([128, 8 * 192], F16, tag="t192", bufs=2)
        t384v = t384[:].rearrange("p (c h f) -> p c h f", c=8, h=2)
        nc.vector.tensor_add(t192[:].rearrange("p (c f) -> p c f", c=8),
                             t384v[:, :, 0], t384v[:, :, 1])
        t192v = t192[:].rearrange("p (c g f) -> p c g f", c=8, g=3)
        xs2 = work.tile([128, 8 * 64], F16, tag="xs2", bufs=2)
        xs2v = xs2[:].rearrange("p (c f) -> p c f", c=8)
        nc.vector.tensor_add(xs2v, t192v[:, :, 0], t192v[:, :, 1])
        xs4 = work.tile([128, 8 * 64], F16, tag="xs4", bufs=2)
        nc.vector.tensor_add(xs4[:].rearrange("p (c f) -> p c f", c=8),
                             xs2v, t192v[:, :, 2])
        return xs4

    def build_xsT(xs4):
        # paired transpose: each [128, 128] block holds chunks 2q (rows
        # 0:64) and 2q+1 (rows 64:128) -> xsT2 [128, (q, n)] via one copy
        ps_x = psum.tile([128, 512], F16, tag="ps_x", bufs=2)
        for q in range(4):
            nc.tensor.transpose(ps_x[:, q * 128:(q + 1) * 128],
                                xs4[:, q * 128:(q + 1) * 128], eye_sb[:])
        xsT2 = work.tile([128, 512], F16, tag="xsT", bufs=2)
        nc.scalar.copy(xsT2[:], ps_x[:])
        return xsT2

    def mm1_exp(xsT2, tp):
        pair = []
        for par in range(2):
            b = 64 * par
            ps_log = psum.tile([128, 512], F32, tag="logits", bufs=2)
            nc.tensor.matmul(ps_log[:],
                             mt2h_sb[b:b + 64, tp * 128:(tp + 1) * 128],
                             xsT2[b:b + 64, :], start=True, stop=True)
            ex = work.tile([128, 512], BF16, tag="exp", bufs=24)
            nc.scalar.activation(ex[:], ps_log[:],
                                 mybir.ActivationFunctionType.Exp,
                                 bias=zbias[:])
            pair.append(ex)
        return pair

    def chunk_mm2(exps, c):
        # chunk c lives at parity c%2, block c//2 of the exp tiles
        q, par = divmod(c, 2)
        ps_val = psum.tile([128, 1024], F32, tag="val", bufs=2)
        for tp in range(6):
            off = 512 * (tp // 3) + 130 * (tp % 3)
            nc.tensor.matmul(ps_val[:, off:off + 130],
                             exps[(tp, par)][:, q * 128:(q + 1) * 128],
                             mbd_sb[:, tp * 130:(tp + 1) * 130],
                             start=True, stop=True)
        sums_ap = (ps_val[:].rearrange("p (h r) -> p h r", h=2)
                   [:, :, 0:390]
                   .rearrange("p h (a r) -> p h a r", a=3)
                   [:, :, :, 128:130])
        rec = work.tile([128, 12], F32, tag="rec", bufs=4)
        nc.vector.reciprocal(
            rec[:].rearrange("p (h a t) -> p h a t", h=2, a=3), sums_ap)
        return ps_val, rec

    def chunk_norm(ps_val, rec, vn, c):
        for h in range(2):
            in0 = (ps_val[:, 512 * h:512 * h + 390]
                   .rearrange("p (a r) -> p a r", a=3)
                   [:, :, 0:128]
                   .rearrange("p a (t d) -> p a t d", t=2))
            in1 = (rec[:, 6 * h:6 * h + 6]
                   .rearrange("p (a t) -> p a t", a=3)
                   .unsqueeze(3)
                   .broadcast_to([128, 3, 2, D]))
            outp = (vn[:, c * 768 + 384 * h:c * 768 + 384 * h + 384]
                    .rearrange("p (a t d) -> p a t d", a=3, t=2))
            nc.vector.tensor_mul(outp, in0, in1)

    def chunk_norm_off(ps_val, rec, vn, c):
        # offloaded normalize: ACT evacuates PSUM -> fp16 SBUF, gpsimd
        # (otherwise idle) does the broadcast multiply, freeing DVE.
        # bf16: unnormalized values can reach ~e^30, far beyond fp16 range
        vv = work.tile([128, 780], BF16, tag="vv", bufs=2)
        for h in range(2):
            nc.scalar.copy(vv[:, 390 * h:390 * h + 390],
                           ps_val[:, 512 * h:512 * h + 390])
            in0 = (vv[:, 390 * h:390 * h + 390]
                   .rearrange("p (a r) -> p a r", a=3)
                   [:, :, 0:128]
                   .rearrange("p a (t d) -> p a t d", t=2))
            in1 = (rec[:, 6 * h:6 * h + 6]
                   .rearrange("p (a t) -> p a t", a=3)
                   .unsqueeze(3)
                   .broadcast_to([128, 3, 2, D]))
            outp = (vn[:, c * 768 + 384 * h:c * 768 + 384 * h + 384]
                    .rearrange("p (a t d) -> p a t d", a=3, t=2))
            nc.gpsimd.tensor_mul(outp, in0, in1)

    def store(it, vn):
        # 1.5 MB store on the ACT HWDGE ring (loads use the sync ring)
        nc.scalar.dma_start(
            out=out[1024 * it:1024 * it + 1024, :]
                .rearrange("(p c) f -> p c f", c=8),
            in_=vn[:].rearrange("p (c f) -> p c f", c=8),
        )

    # -------- prologue: iteration 0's xsT, loads for 0 and 1 --------
    xts = {0: load(0)}
    if NIT > 1:
        xts[1] = load(1)
    xsT2 = build_xsT(tree_rest(tree_l1(xts[0])))

    exps_prev = None
    vn_prev = None
    for it in range(NIT + 1):
        if it + 2 < NIT:
            xts[it + 2] = load(it + 2)
        t384n = tree_l1(xts.pop(it + 1)) if it + 1 < NIT else None
        exps = {}
        vn = None
        if it < NIT:
            vn = work.tile([128, 8 * T * D], F16, tag="vn", bufs=2)
        xs4n = None
        xsT2_next = None
        # interleave this iteration's mm1/exp pairs with the previous
        # iteration's mm2 chunks so PE never idles on ACT's exp pace; the
        # next iteration's tree and transposes are woven in mid-body.
        for tp in range(6):
            if it < NIT:
                exps[(tp, 0)], exps[(tp, 1)] = mm1_exp(xsT2, tp)
            if it > 0:
                pv, rec = chunk_mm2(exps_prev, tp)
                chunk_norm(pv, rec, vn_prev, tp)
            if tp == 2 and t384n is not None:
                xs4n = tree_rest(t384n)
            if tp == 4 and xs4n is not None:
                xsT2_next = build_xsT(xs4n)
        if it > 0:
            for c in (6, 7):
                pv, rec = chunk_mm2(exps_prev, c)
                chunk_norm_off(pv, rec, vn_prev, c)
            store(it - 1, vn_prev)
        if xsT2_next is not None:
            xsT2 = xsT2_next
        exps_prev, vn_prev = exps, vn


_NC_CACHE = {}


def build_nc():
    if "nc" in _NC_CACHE:
        return _NC_CACHE["nc"]
    nc = bacc.Bacc("TRN2", target_bir_lowering=False, debug=False,
                   num_devices=NCORES)
    # x is pre-transposed on the host to [BS, N, L, D], n-padded to 896 rows
    # per batch with zeros, flattened to [7168, 12, 64] and cast fp16. The
    # output is produced padded as [7168, (t d)] fp16; the host slices off
    # the 13 pad rows per batch and upcasts.
    x_ap = nc.dram_tensor("x_sh", [ROWS, L, D], F16, kind="ExternalInput").ap()
    mt2h_ap = nc.dram_tensor("mt2h", [128, 6 * 128], F16, kind="ExternalInput").ap()
    mbd_ap = nc.dram_tensor("mbd", [128, 6 * 130], BF16, kind="ExternalInput").ap()
    eye_ap = nc.dram_tensor("eye", [128, 128], F16, kind="ExternalInput").ap()
    out_ap = nc.dram_tensor("out", [ROWS, T * D], F16, kind="ExternalOutput").ap()
    with tile.TileContext(nc) as tc:
        kernel_body(tc, out_ap, x_ap, mt2h_ap, mbd_ap, eye_ap)
    nc.compile()
    _NC_CACHE["nc"] = nc
    return nc


def make_in_maps(x, M):
    import ml_dtypes
    x = np.asarray(x, dtype=np.float32)
    mt2h, mbd, eye = build_consts(M)
    mbd_bf = mbd.astype(ml_dtypes.bfloat16)
    maps = []
    for i in range(NCORES):
        xp = np.zeros((BS, NPAD, L, D), np.float16)
        xp[:, :N] = x[i * BS:(i + 1) * BS].transpose(0, 2, 1, 3).astype(np.float16)
        maps.append({"x_sh": xp.reshape(ROWS, L, D),
                     "mt2h": mt2h, "mbd": mbd_bf, "eye": eye})
    return maps


def gather_outputs(res):
    outs = []
    for i in range(NCORES):
        o = np.asarray(res[i]["out"], dtype=np.float32)
        o = o.reshape(BS, NPAD, T, D)[:, :N].transpose(0, 2, 1, 3)
        outs.append(o)
    return np.ascontiguousarray(np.concatenate(outs, axis=0))


def kernel(x, M):
    nc = build_nc()
    in_maps = make_in_maps(x, M)
    res = run_bass_kernel_spmd(nc, in_maps, list(range(NCORES))).results
    return gather_outputs(res)


if __name__ == "__main__":
    rng = np.random.default_rng(0)
    x = rng.standard_normal((B, L, N, D), dtype=np.float32)
    M = (rng.standard_normal((T, MNUM, D), dtype=np.float32) * 0.125).astype(np.float32)
    out = kernel(x, M)
    print("out", out.shape, out.dtype, float(np.abs(out).max()))
